# revision 25
# baseline (speedup 1.0000x reference)
"""MixerBlock kernel for Trainium2 (8 NeuronCores, data-parallel over batch).

Reference computation (per batch b of x[B,T,H], B=32, T=H=1024):
  y   = LN1(x)                                    # over H
  u1  = gelu(W1m @ y + tb1 x 1)    W1m = tril*tw1 # temporal mix in [T,H] layout
  x2  = x + W2m @ u1 + tb2 x 1     W2m = tril*tw2
  y2  = LN2(x2)
  v1  = gelu(cw1' @ y2^T + cb1' x 1)              # [H,T] layout, LN2 g/b folded
  out = x2 + (v1^T' @ cw2T) + 1 x cb2             # back in [T,H]

Matmul operands are bf16 (1 PE cycle/row, tolerance is 2e-2 and bf16 lands
~1e-3). w1/w2 are tril-packed (36 of 64 k-tiles) and, with cw1, persistent
in SBUF; cw2 streams double-buffered per batch. x loads are issued on the
Activation engine's HW-DGE queue so they never sit behind output stores in
the SP queue, and each batch's LN1 stats+apply are hoisted into the prior
batch's channel phase to keep the PE fed across batch/rep boundaries.
"""
import os
import numpy as np
import ml_dtypes
from contextlib import ExitStack

import concourse.bass as bass
import concourse.tile as tile
from concourse import bacc, mybir
from concourse.bass_utils import run_bass_kernel_spmd
from concourse.masks import make_identity

_bf16 = ml_dtypes.bfloat16
F32 = mybir.dt.float32
BF16 = mybir.dt.bfloat16
AF = mybir.ActivationFunctionType
ALU = mybir.AluOpType

B, T, H = 32, 1024, 1024
NCORES = 8
BPC = B // NCORES          # batches per core
RT = T // 128              # 8 row tiles
NTRI = RT * (RT + 1) // 2  # 36 lower-triangular k-tiles
TRI = [m * (m + 1) // 2 for m in range(RT)]
LN_EPS = 1e-5


def build(apply_g1=False, apply_b1=False, time_reps=1, bpc=BPC,
          cmm1_groups=2, unroll_reps=False, body_reps=1):
    nc = bacc.Bacc("TRN2", target_bir_lowering=False, debug=False,
                   num_devices=NCORES)
    x_d = nc.dram_tensor("x", [bpc, 128, RT, H], F32, kind="ExternalInput").ap()
    w1_d = nc.dram_tensor("w1", [128, NTRI, 128], BF16, kind="ExternalInput").ap()
    w2_d = nc.dram_tensor("w2", [128, NTRI, 128], BF16, kind="ExternalInput").ap()
    cw1_d = nc.dram_tensor("cw1", [128, RT * RT, 128], BF16, kind="ExternalInput").ap()
    cw2_d = nc.dram_tensor("cw2", [128, RT, H], BF16, kind="ExternalInput").ap()
    tb1_d = nc.dram_tensor("tb1", [128, RT], F32, kind="ExternalInput").ap()
    tb2_d = nc.dram_tensor("tb2", [128, RT], F32, kind="ExternalInput").ap()
    cb1_d = nc.dram_tensor("cb1", [128, RT], F32, kind="ExternalInput").ap()
    cb2_d = nc.dram_tensor("cb2", [H], F32, kind="ExternalInput").ap()
    g1_d = nc.dram_tensor("g1", [H], F32, kind="ExternalInput").ap()
    b1_d = nc.dram_tensor("b1", [H], F32, kind="ExternalInput").ap()
    out_d = nc.dram_tensor("out", [bpc, 128, RT, H], F32, kind="ExternalOutput").ap()

    def bcast(ap_1d, n):
        return bass.AP(tensor=ap_1d.tensor, offset=ap_1d.offset,
                       ap=[[0, 128], [1, n]])

    with tile.TileContext(nc) as tc:
        with ExitStack() as ctx:
            singles = ctx.enter_context(tc.tile_pool(name="singles", bufs=1))
            xp = ctx.enter_context(tc.tile_pool(name="xp", bufs=2))
            abp = ctx.enter_context(tc.tile_pool(name="abp", bufs=2))
            prep = ctx.enter_context(tc.tile_pool(name="prep", bufs=2))
            otp = ctx.enter_context(tc.tile_pool(name="otp", bufs=4))
            cw2p = ctx.enter_context(tc.tile_pool(name="cw2p", bufs=1))
            stats = ctx.enter_context(tc.tile_pool(name="stats", bufs=2))
            psum = ctx.enter_context(tc.tile_pool(name="psum", bufs=8, space="PSUM"))

            # persistent weights + constants
            w1_sb = singles.tile([128, NTRI, 128], BF16)
            w2_sb = singles.tile([128, NTRI, 128], BF16)
            cw1_sb = singles.tile([128, RT * RT, 128], BF16)
            tb1_sb = singles.tile([128, RT], F32)
            tb2_sb = singles.tile([128, RT], F32)
            cb1_sb = singles.tile([128, RT], F32)
            cb2_sb = singles.tile([128, H], F32)
            eps_sb = singles.tile([128, 1], F32)
            ident = singles.tile([128, 128], BF16)
            nc.sync.dma_start(w1_sb[:], w1_d[:])
            nc.sync.dma_start(w2_sb[:], w2_d[:])
            nc.sync.dma_start(cw1_sb[:], cw1_d[:])
            nc.sync.dma_start(tb1_sb[:], tb1_d[:])
            nc.sync.dma_start(tb2_sb[:], tb2_d[:])
            nc.sync.dma_start(cb1_sb[:], cb1_d[:])
            nc.sync.dma_start(cb2_sb[:], bcast(cb2_d, H))
            nc.vector.memset(eps_sb[:], LN_EPS)
            make_identity(nc, ident[:])
            if apply_g1:
                g1_sb = singles.tile([128, H], F32)
                nc.sync.dma_start(g1_sb[:], bcast(g1_d, H))
            if apply_b1:
                b1_sb = singles.tile([128, H], F32)
                nc.sync.dma_start(b1_sb[:], bcast(b1_d, H))

            def ln_rs_m2(x_slice, rs_out, m2_out, neg=False):
                """layer-norm stats for [128, H] slice -> rs, m2 [128,1]
                such that normalized = x*rs - m2 (or x*rs + m2 if neg)"""
                st = stats.tile([128, 2, 6], F32, name="st", tag="st", bufs=4)
                nc.vector.bn_stats(st[:, 0, :], x_slice[:, 0:512])
                nc.vector.bn_stats(st[:, 1, :], x_slice[:, 512:1024])
                mv = stats.tile([128, 2], F32, name="mv", tag="mv", bufs=4)
                nc.vector.bn_aggr(mv[:], st[:])
                sd = stats.tile([128, 1], F32, name="sd", tag="sd", bufs=4)
                nc.scalar.activation(sd[:], mv[:, 1:2], AF.Sqrt, bias=eps_sb[:])
                nc.vector.reciprocal(rs_out, sd[:])
                if neg:
                    nc.vector.tensor_scalar(
                        out=m2_out, in0=mv[:, 0:1], scalar1=rs_out,
                        scalar2=-1.0, op0=ALU.mult, op1=ALU.mult)
                else:
                    nc.vector.tensor_mul(m2_out, mv[:, 0:1], rs_out)

            def load_x(b, eng=None):
                """start x[b] DMA per r-chunk. Boundary loads go on the
                Activation HW-DGE queue (needed immediately, and its WAR wait
                is already satisfied); hoisted loads go on SP so the Act
                sequencer never blocks on the x-buffer WAR wait."""
                eng = eng or nc.sync
                x_sb = xp.tile([128, RT, H], F32, name="x_sb", tag="x")
                for r in range(RT):
                    eng.dma_start(x_sb[:, r, :], x_d[b][:, r, :])
                return x_sb

            def stats_apply(x_sb):
                """LN1 stats + apply -> y_sb (bf16), per r so work pipelines
                behind the arriving x chunks"""
                rs1 = stats.tile([128, RT], F32, name="rs1", tag="rs1")
                m21 = stats.tile([128, RT], F32, name="m21", tag="m21")
                y_sb = abp.tile([128, RT, H], BF16, name="y_sb", tag="A")
                for r in range(RT):
                    ln_rs_m2(x_sb[:, r, :], rs1[:, r:r + 1], m21[:, r:r + 1])
                    nc.vector.tensor_scalar(
                        out=y_sb[:, r, :], in0=x_sb[:, r, :],
                        scalar1=rs1[:, r:r + 1], scalar2=m21[:, r:r + 1],
                        op0=ALU.mult, op1=ALU.subtract)
                    if apply_g1:
                        nc.vector.tensor_mul(y_sb[:, r, :], y_sb[:, r, :], g1_sb[:])
                    if apply_b1:
                        nc.vector.tensor_add(y_sb[:, r, :], y_sb[:, r, :], b1_sb[:])
                return y_sb

            pending = [None]

            def batch_body(b):
                x_sb, y_sb = pending[0]
                if b + 1 < bpc:
                    x_next = load_x(b + 1)

                # ---- temporal MM1 + gelu -> u1g ----
                u1g = abp.tile([128, RT, H], BF16, name="u1g", tag="B")
                for m in range(RT):
                    pns = psum.tile([128, H], F32, name="pns", tag="ps", bufs=2)
                    for k in range(m + 1):
                        for n in range(2):
                            nc.tensor.matmul(
                                pns[:, 512 * n:512 * n + 512],
                                w1_sb[:, TRI[m] + k, :],
                                y_sb[:, k, 512 * n:512 * n + 512],
                                start=(k == 0), stop=(k == m))
                    nc.scalar.activation(
                        u1g[:, m, :], pns[:],
                        AF.Gelu, bias=tb1_sb[:, m:m + 1])

                # ---- temporal MM2 + bias + residual -> x_sb; LN2 stats ----
                # interleaved with LN2-apply + PE transpose so the DVE
                # stt/stats chain (longer than the PE matmul time in this
                # phase) gets runway before the transposes need y2pre
                rs2 = stats.tile([128, RT], F32, name="rs2", tag="rs2")
                m22 = stats.tile([128, RT], F32, name="m22", tag="m22")
                y2T = abp.tile([128, RT, RT, 128], BF16, name="y2T", tag="B")

                def t_mm2(m):
                    pns = psum.tile([128, H], F32, name="pns", tag="ps", bufs=2)
                    for k in range(m + 1):
                        for n in range(2):
                            nc.tensor.matmul(
                                pns[:, 512 * n:512 * n + 512],
                                w2_sb[:, TRI[m] + k, :],
                                u1g[:, k, 512 * n:512 * n + 512],
                                start=(k == 0), stop=(k == m))
                    nc.vector.scalar_tensor_tensor(
                        out=x_sb[:, m, :], in0=pns[:],
                        scalar=tb2_sb[:, m:m + 1], in1=x_sb[:, m, :],
                        op0=ALU.add, op1=ALU.add)
                    ln_rs_m2(x_sb[:, m, :], rs2[:, m:m + 1], m22[:, m:m + 1],
                             neg=True)

                def transp(r):
                    # y2T[p, r, c, j] = y2[t = 128*r + j, h = 128*c + p]
                    # LN2 apply on Act (DVE is the tight engine this phase)
                    y2pre = prep.tile([128, H], BF16, name="y2pre", tag="pre")
                    nc.scalar.activation(
                        y2pre[:], x_sb[:, r, :], AF.Identity,
                        bias=m22[:, r:r + 1], scale=rs2[:, r:r + 1])
                    tp = psum.tile([128, H], BF16, name="tp", tag="tp", bufs=2)
                    for c in range(RT):
                        nc.tensor.transpose(
                            tp[:, 128 * c:128 * c + 128],
                            y2pre[:, 128 * c:128 * c + 128], ident[:])
                    nc.scalar.copy(
                        y2T[:, r, :, :],
                        tp[:].rearrange("p (a b) -> p a b", a=RT))

                # ---- channel MM1: free-dim group g of width 1024//CG needs
                # only y2T rows (RT//CG)*g.., so groups weave between
                # transpose pairs and keep the PE busy while the DVE
                # stats/y2pre tail drains
                v1g = abp.tile([128, RT, H], BF16, name="v1g", tag="A")
                CG = cmm1_groups  # 1 (flat), 2, or 4
                GW = 1024 // CG   # free width per group
                GR = RT // CG     # y2T rows per group

                def c_mm1(g):
                    for mo in range(RT):
                        if CG == 1:
                            pgs = [psum.tile([128, 512], F32, name="pns4",
                                             tag="c1", bufs=2) for _ in range(2)]
                            for kh in range(RT):
                                for n in range(2):
                                    nc.tensor.matmul(
                                        pgs[n][:], cw1_sb[:, mo * RT + kh, :],
                                        y2T[:, 4 * n:4 * n + 4, kh, :],
                                        start=(kh == 0), stop=(kh == RT - 1))
                            for n in range(2):
                                nc.scalar.activation(
                                    v1g[:, mo, 512 * n:512 * n + 512], pgs[n][:],
                                    AF.Gelu, bias=cb1_sb[:, mo:mo + 1])
                            continue
                        pg = psum.tile([128, GW], F32, name="pns4", tag="c1",
                                       bufs=2)
                        for kh in range(RT):
                            nc.tensor.matmul(
                                pg[:], cw1_sb[:, mo * RT + kh, :],
                                y2T[:, GR * g:GR * g + GR, kh, :],
                                start=(kh == 0), stop=(kh == RT - 1))
                        nc.scalar.activation(
                            v1g[:, mo, GW * g:GW * g + GW], pg[:],
                            AF.Gelu, bias=cb1_sb[:, mo:mo + 1])

                def cw2_prefetch():
                    cw2t = cw2p.tile([128, RT, H], BF16, name="cw2t", tag="cw2")
                    nc.sync.dma_start(cw2t[:, :, 0:512], cw2_d[:, :, 0:512])
                    nc.sync.dma_start(cw2t[:, :, 512:1024], cw2_d[:, :, 512:1024])
                    return cw2t

                if CG == 4:
                    for m in range(7):
                        t_mm2(m)
                    cw2t = cw2_prefetch()
                    transp(0); transp(1)
                    t_mm2(7)
                    c_mm1(0)
                    transp(2); transp(3)
                    c_mm1(1)
                    transp(4); transp(5)
                    c_mm1(2)
                    transp(6); transp(7)
                    c_mm1(3)
                elif CG == 2:
                    for m in range(6):
                        t_mm2(m)
                    cw2t = cw2_prefetch()
                    transp(0); transp(1)
                    t_mm2(6)
                    transp(2); transp(3)
                    t_mm2(7)
                    c_mm1(0)
                    transp(4); transp(5); transp(6); transp(7)
                    c_mm1(1)
                else:
                    for m in range(6):
                        t_mm2(m)
                    cw2t = cw2_prefetch()
                    transp(0); transp(1)
                    t_mm2(6)
                    transp(2); transp(3)
                    t_mm2(7)
                    for r in range(4, RT):
                        transp(r)
                    c_mm1(0)

                # hoist next batch's LN1 stats+apply into this channel phase
                if b + 1 < bpc:
                    y_next = stats_apply(x_next)
                    pending[0] = (x_next, y_next)

                # ---- channel MM2 + bias + residual -> out ----
                for mt in range(RT):
                    pns = psum.tile([128, H], F32, name="pns", tag="ps", bufs=2)
                    for ko in range(RT):
                        for n in range(2):
                            nc.tensor.matmul(
                                pns[:, 512 * n:512 * n + 512],
                                v1g[:, ko, 128 * mt:128 * mt + 128],
                                cw2t[:, ko, 512 * n:512 * n + 512],
                                start=(ko == 0), stop=(ko == RT - 1))
                    o_t = otp.tile([128, H], F32, name="o_t", tag="o")
                    nc.vector.scalar_tensor_tensor(
                        out=o_t[:], in0=pns[:], scalar=1.0,
                        in1=x_sb[:, mt, :], op0=ALU.mult, op1=ALU.add)
                    nc.gpsimd.tensor_add(o_t[:], o_t[:], cb2_sb[:])
                    nc.sync.dma_start(out_d[b][:, mt, :], o_t[:])

            def rep_body():
                x0 = load_x(0, eng=nc.scalar)
                y0 = stats_apply(x0)
                pending[0] = (x0, y0)
                for b in range(bpc):
                    batch_body(b)

            if time_reps > 1 and not unroll_reps:
                assert time_reps % body_reps == 0
                with tc.For_i(0, time_reps // body_reps, 1,
                              hint_engines=(mybir.EngineType.PE,
                                            mybir.EngineType.DVE,
                                            mybir.EngineType.Activation,
                                            mybir.EngineType.SP,
                                            mybir.EngineType.Pool)):
                    for _ in range(body_reps):
                        rep_body()
            else:
                for _ in range(time_reps):
                    rep_body()

    nc.compile()
    return nc


def prep_inputs(x, tw1, tb1, tw2, tb2, cw1, cb1, cw2, cb2,
                ln1_g, ln1_b, ln2_g, ln2_b):
    """Host-side layout + weight folding. Returns (in_maps, apply_g1, apply_b1)."""
    f = np.float32
    x = np.ascontiguousarray(np.asarray(x, f))
    mask = np.tril(np.ones((T, T), f))
    w1mT = (mask * np.asarray(tw1, f)).T          # [j, i]
    w2mT = (mask * np.asarray(tw2, f)).T
    cw1 = np.asarray(cw1, f)
    cw2 = np.asarray(cw2, f)
    ln2_g = np.asarray(ln2_g, f)
    ln2_b = np.asarray(ln2_b, f)
    # fold LN2 affine into channel MLP first layer
    cw1p = cw1 * ln2_g[None, :]                   # [o, h]
    cb1p = np.asarray(cb1, f) + cw1 @ ln2_b       # [o]
    cw1pT = cw1p.T                                # [h, o]
    cw2T = cw2.T                                  # [o, p]

    def tiles4(w):   # [1024,1024] -> [128, 8(k), 8(m), 128] (p=row%128)
        return w.reshape(RT, 128, RT, 128).transpose(1, 0, 2, 3)

    def tri_pack(w):  # -> [128, 36, 128] bf16, slot TRI[m]+k for k<=m
        t4 = tiles4(w)
        return np.ascontiguousarray(np.stack(
            [t4[:, k, m] for m in range(RT) for k in range(m + 1)],
            axis=1).astype(_bf16))

    def full_pack(w):  # -> [128, 64, 128] bf16, slot mo*8+kh
        t4 = tiles4(w)
        return np.ascontiguousarray(np.stack(
            [t4[:, kh, mo] for mo in range(RT) for kh in range(RT)],
            axis=1).astype(_bf16))

    def tiles3(w):   # [1024,1024] -> [128, 8, 1024]
        return np.ascontiguousarray(
            w.reshape(RT, 128, H).transpose(1, 0, 2).astype(_bf16))

    def bias_t(v):   # [1024] -> [128, 8]
        return np.ascontiguousarray(np.asarray(v, f).reshape(RT, 128).T)

    g1 = np.asarray(ln1_g, f)
    b1 = np.asarray(ln1_b, f)
    apply_g1 = not np.all(g1 == 1.0)
    apply_b1 = not np.all(b1 == 0.0)

    shared = {
        "w1": tri_pack(w1mT), "w2": tri_pack(w2mT),
        "cw1": full_pack(cw1pT), "cw2": tiles3(cw2T),
        "tb1": bias_t(tb1), "tb2": bias_t(tb2), "cb1": bias_t(cb1p),
        "cb2": np.ascontiguousarray(np.asarray(cb2, f)),
        "g1": np.ascontiguousarray(g1), "b1": np.ascontiguousarray(b1),
    }
    # x: [B,T,H] -> per-core [BPC, 128, RT, H]  (t = r*128 + p)
    xs = x.reshape(NCORES, BPC, RT, 128, H).transpose(0, 1, 3, 2, 4)
    in_maps = [{"x": np.ascontiguousarray(xs[c]), **shared}
               for c in range(NCORES)]
    return in_maps, apply_g1, apply_b1


_cache = {}


def kernel(**inputs) -> np.ndarray:
    in_maps, apply_g1, apply_b1 = prep_inputs(**inputs)
    key = (apply_g1, apply_b1)
    if key not in _cache:
        _cache[key] = build(apply_g1=apply_g1, apply_b1=apply_b1, time_reps=1)
    nc = _cache[key]
    res = run_bass_kernel_spmd(nc, in_maps, list(range(NCORES)))
    # out per core: [BPC, 128, RT, H] -> [BPC, T, H]
    outs = [r["out"].transpose(0, 2, 1, 3).reshape(BPC, T, H)
            for r in res.results]
    return np.ascontiguousarray(np.concatenate(outs, axis=0), dtype=np.float32)


# revision 28
# speedup vs baseline: 1.0529x; 1.0529x over previous
"""MixerBlock kernel for Trainium2 (8 NeuronCores, data-parallel over batch).

Reference computation (per batch b of x[B,T,H], B=32, T=H=1024):
  y   = LN1(x)                                    # over H
  u1  = gelu(W1m @ y + tb1 x 1)    W1m = tril*tw1 # temporal mix in [T,H] layout
  x2  = x + W2m @ u1 + tb2 x 1     W2m = tril*tw2
  y2  = LN2(x2)
  v1  = gelu(cw1' @ y2^T + cb1' x 1)              # [H,T] layout, LN2 g/b folded
  out = x2 + (v1^T' @ cw2T) + 1 x cb2             # back in [T,H]

Matmul operands are bf16 (1 PE cycle/row, tolerance is 2e-2 and bf16 lands
~1e-3). w1/w2 are tril-packed (36 of 64 k-tiles) and, with cw1, persistent
in SBUF; cw2 streams double-buffered per batch. x loads are issued on the
Activation engine's HW-DGE queue so they never sit behind output stores in
the SP queue, and each batch's LN1 stats+apply are hoisted into the prior
batch's channel phase to keep the PE fed across batch/rep boundaries.
"""
import os
import numpy as np
import ml_dtypes
from contextlib import ExitStack

import concourse.bass as bass
import concourse.tile as tile
from concourse import bacc, mybir
from concourse.bass_utils import run_bass_kernel_spmd
from concourse.masks import make_identity

_bf16 = ml_dtypes.bfloat16
F32 = mybir.dt.float32
BF16 = mybir.dt.bfloat16
AF = mybir.ActivationFunctionType
ALU = mybir.AluOpType

B, T, H = 32, 1024, 1024
NCORES = 8
BPC = B // NCORES          # batches per core
RT = T // 128              # 8 row tiles
NTRI = RT * (RT + 1) // 2  # 36 lower-triangular k-tiles
TRI = [m * (m + 1) // 2 for m in range(RT)]
LN_EPS = 1e-5


def build(apply_g1=False, apply_b1=False, time_reps=1, bpc=BPC,
          cmm1_groups=2, unroll_reps=False, body_reps=1):
    nc = bacc.Bacc("TRN2", target_bir_lowering=False, debug=False,
                   num_devices=NCORES)
    x_d = nc.dram_tensor("x", [bpc, 128, RT, H], BF16, kind="ExternalInput").ap()
    w1_d = nc.dram_tensor("w1", [128, NTRI, 128], BF16, kind="ExternalInput").ap()
    w2_d = nc.dram_tensor("w2", [128, NTRI, 128], BF16, kind="ExternalInput").ap()
    cw1_d = nc.dram_tensor("cw1", [128, RT * RT, 128], BF16, kind="ExternalInput").ap()
    cw2_d = nc.dram_tensor("cw2", [128, RT, H], BF16, kind="ExternalInput").ap()
    tb1_d = nc.dram_tensor("tb1", [128, RT], F32, kind="ExternalInput").ap()
    tb2_d = nc.dram_tensor("tb2", [128, RT], F32, kind="ExternalInput").ap()
    cb1_d = nc.dram_tensor("cb1", [128, RT], F32, kind="ExternalInput").ap()
    cb2_d = nc.dram_tensor("cb2", [H], F32, kind="ExternalInput").ap()
    g1_d = nc.dram_tensor("g1", [H], F32, kind="ExternalInput").ap()
    b1_d = nc.dram_tensor("b1", [H], F32, kind="ExternalInput").ap()
    out_d = nc.dram_tensor("out", [bpc, 128, RT, H], F32, kind="ExternalOutput").ap()

    def bcast(ap_1d, n):
        return bass.AP(tensor=ap_1d.tensor, offset=ap_1d.offset,
                       ap=[[0, 128], [1, n]])

    with tile.TileContext(nc) as tc:
        with ExitStack() as ctx:
            singles = ctx.enter_context(tc.tile_pool(name="singles", bufs=1))
            xp = ctx.enter_context(tc.tile_pool(name="xp", bufs=2))
            abp = ctx.enter_context(tc.tile_pool(name="abp", bufs=2))
            prep = ctx.enter_context(tc.tile_pool(name="prep", bufs=2))
            otp = ctx.enter_context(tc.tile_pool(name="otp", bufs=4))
            stats = ctx.enter_context(tc.tile_pool(name="stats", bufs=2))
            psum = ctx.enter_context(tc.tile_pool(name="psum", bufs=8, space="PSUM"))

            # persistent weights + constants
            w1_sb = singles.tile([128, NTRI, 128], BF16)
            w2_sb = singles.tile([128, NTRI, 128], BF16)
            cw1_sb = singles.tile([128, RT * RT, 128], BF16)
            cw2_sb = singles.tile([128, RT, H], BF16)
            tb1_sb = singles.tile([128, RT], F32)
            tb2_sb = singles.tile([128, RT], F32)
            cb1_sb = singles.tile([128, RT], F32)
            cb2_sb = singles.tile([128, H], F32)
            eps_sb = singles.tile([128, 1], F32)
            ident = singles.tile([128, 128], BF16)
            nc.sync.dma_start(w1_sb[:], w1_d[:])
            nc.sync.dma_start(w2_sb[:], w2_d[:])
            nc.sync.dma_start(cw1_sb[:], cw1_d[:])
            nc.sync.dma_start(cw2_sb[:, :, 0:512], cw2_d[:, :, 0:512])
            nc.sync.dma_start(cw2_sb[:, :, 512:1024], cw2_d[:, :, 512:1024])
            nc.sync.dma_start(tb1_sb[:], tb1_d[:])
            nc.sync.dma_start(tb2_sb[:], tb2_d[:])
            nc.sync.dma_start(cb1_sb[:], cb1_d[:])
            nc.sync.dma_start(cb2_sb[:], bcast(cb2_d, H))
            nc.vector.memset(eps_sb[:], LN_EPS)
            make_identity(nc, ident[:])
            if apply_g1:
                g1_sb = singles.tile([128, H], F32)
                nc.sync.dma_start(g1_sb[:], bcast(g1_d, H))
            if apply_b1:
                b1_sb = singles.tile([128, H], F32)
                nc.sync.dma_start(b1_sb[:], bcast(b1_d, H))

            def ln_rs_m2(x_slice, rs_out, m2_out, neg=False):
                """layer-norm stats for [128, H] slice -> rs, m2 [128,1]
                such that normalized = x*rs - m2 (or x*rs + m2 if neg)"""
                st = stats.tile([128, 2, 6], F32, name="st", tag="st", bufs=4)
                nc.vector.bn_stats(st[:, 0, :], x_slice[:, 0:512])
                nc.vector.bn_stats(st[:, 1, :], x_slice[:, 512:1024])
                mv = stats.tile([128, 2], F32, name="mv", tag="mv", bufs=4)
                nc.vector.bn_aggr(mv[:], st[:])
                sd = stats.tile([128, 1], F32, name="sd", tag="sd", bufs=4)
                nc.scalar.activation(sd[:], mv[:, 1:2], AF.Sqrt, bias=eps_sb[:])
                nc.vector.reciprocal(rs_out, sd[:])
                if neg:
                    nc.vector.tensor_scalar(
                        out=m2_out, in0=mv[:, 0:1], scalar1=rs_out,
                        scalar2=-1.0, op0=ALU.mult, op1=ALU.mult)
                else:
                    nc.vector.tensor_mul(m2_out, mv[:, 0:1], rs_out)

            def load_x(b, eng=None):
                """start x[b] DMA per r-chunk. Boundary loads go on the
                Activation HW-DGE queue (needed immediately, and its WAR wait
                is already satisfied); hoisted loads go on SP so the Act
                sequencer never blocks on the x-buffer WAR wait."""
                eng = eng or nc.sync
                x_sb = xp.tile([128, RT, H], BF16, name="x_sb", tag="x")
                for r in range(RT):
                    eng.dma_start(x_sb[:, r, :], x_d[b][:, r, :])
                return x_sb

            def stats_apply(x_sb):
                """LN1 stats + apply -> y_sb (bf16), per r so work pipelines
                behind the arriving x chunks"""
                rs1 = stats.tile([128, RT], F32, name="rs1", tag="rs1")
                m21 = stats.tile([128, RT], F32, name="m21", tag="m21")
                y_sb = abp.tile([128, RT, H], BF16, name="y_sb", tag="A")
                for r in range(RT):
                    ln_rs_m2(x_sb[:, r, :], rs1[:, r:r + 1], m21[:, r:r + 1])
                    nc.vector.tensor_scalar(
                        out=y_sb[:, r, :], in0=x_sb[:, r, :],
                        scalar1=rs1[:, r:r + 1], scalar2=m21[:, r:r + 1],
                        op0=ALU.mult, op1=ALU.subtract)
                    if apply_g1:
                        nc.vector.tensor_mul(y_sb[:, r, :], y_sb[:, r, :], g1_sb[:])
                    if apply_b1:
                        nc.vector.tensor_add(y_sb[:, r, :], y_sb[:, r, :], b1_sb[:])
                return y_sb

            pending = [None]

            def batch_body(b):
                x_sb, y_sb = pending[0]
                if b + 1 < bpc:
                    x_next = load_x(b + 1)

                # ---- temporal MM1 + gelu -> u1g ----
                u1g = abp.tile([128, RT, H], BF16, name="u1g", tag="B")
                for m in range(RT):
                    pns = psum.tile([128, H], F32, name="pns", tag="ps", bufs=2)
                    for k in range(m + 1):
                        for n in range(2):
                            nc.tensor.matmul(
                                pns[:, 512 * n:512 * n + 512],
                                w1_sb[:, TRI[m] + k, :],
                                y_sb[:, k, 512 * n:512 * n + 512],
                                start=(k == 0), stop=(k == m))
                    nc.scalar.activation(
                        u1g[:, m, :], pns[:],
                        AF.Gelu, bias=tb1_sb[:, m:m + 1])

                # ---- temporal MM2 + bias + residual -> x_sb; LN2 stats ----
                # interleaved with LN2-apply + PE transpose so the DVE
                # stt/stats chain (longer than the PE matmul time in this
                # phase) gets runway before the transposes need y2pre
                rs2 = stats.tile([128, RT], F32, name="rs2", tag="rs2")
                m22 = stats.tile([128, RT], F32, name="m22", tag="m22")
                y2T = abp.tile([128, RT, RT, 128], BF16, name="y2T", tag="B")

                def t_mm2(m):
                    pns = psum.tile([128, H], F32, name="pns", tag="ps", bufs=2)
                    for k in range(m + 1):
                        for n in range(2):
                            nc.tensor.matmul(
                                pns[:, 512 * n:512 * n + 512],
                                w2_sb[:, TRI[m] + k, :],
                                u1g[:, k, 512 * n:512 * n + 512],
                                start=(k == 0), stop=(k == m))
                    nc.vector.scalar_tensor_tensor(
                        out=x_sb[:, m, :], in0=pns[:],
                        scalar=tb2_sb[:, m:m + 1], in1=x_sb[:, m, :],
                        op0=ALU.add, op1=ALU.add)
                    ln_rs_m2(x_sb[:, m, :], rs2[:, m:m + 1], m22[:, m:m + 1],
                             neg=True)

                def transp(r):
                    # y2T[p, r, c, j] = y2[t = 128*r + j, h = 128*c + p]
                    # LN2 apply on Act (DVE is the tight engine this phase)
                    y2pre = prep.tile([128, H], BF16, name="y2pre", tag="pre")
                    nc.scalar.activation(
                        y2pre[:], x_sb[:, r, :], AF.Identity,
                        bias=m22[:, r:r + 1], scale=rs2[:, r:r + 1])
                    tp = psum.tile([128, H], BF16, name="tp", tag="tp", bufs=2)
                    for c in range(RT):
                        nc.tensor.transpose(
                            tp[:, 128 * c:128 * c + 128],
                            y2pre[:, 128 * c:128 * c + 128], ident[:])
                    nc.scalar.copy(
                        y2T[:, r, :, :],
                        tp[:].rearrange("p (a b) -> p a b", a=RT))

                # ---- channel MM1: free-dim group g of width 1024//CG needs
                # only y2T rows (RT//CG)*g.., so groups weave between
                # transpose pairs and keep the PE busy while the DVE
                # stats/y2pre tail drains
                v1g = abp.tile([128, RT, H], BF16, name="v1g", tag="A")
                CG = cmm1_groups  # 1 (flat), 2, or 4
                GW = 1024 // CG   # free width per group
                GR = RT // CG     # y2T rows per group

                def c_mm1(g):
                    for mo in range(RT):
                        if CG == 1:
                            pgs = [psum.tile([128, 512], F32, name="pns4",
                                             tag="c1", bufs=2) for _ in range(2)]
                            for kh in range(RT):
                                for n in range(2):
                                    nc.tensor.matmul(
                                        pgs[n][:], cw1_sb[:, mo * RT + kh, :],
                                        y2T[:, 4 * n:4 * n + 4, kh, :],
                                        start=(kh == 0), stop=(kh == RT - 1))
                            for n in range(2):
                                nc.scalar.activation(
                                    v1g[:, mo, 512 * n:512 * n + 512], pgs[n][:],
                                    AF.Gelu, bias=cb1_sb[:, mo:mo + 1])
                            continue
                        pg = psum.tile([128, GW], F32, name="pns4", tag="c1",
                                       bufs=2)
                        for kh in range(RT):
                            nc.tensor.matmul(
                                pg[:], cw1_sb[:, mo * RT + kh, :],
                                y2T[:, GR * g:GR * g + GR, kh, :],
                                start=(kh == 0), stop=(kh == RT - 1))
                        nc.scalar.activation(
                            v1g[:, mo, GW * g:GW * g + GW], pg[:],
                            AF.Gelu, bias=cb1_sb[:, mo:mo + 1])

                if CG == 4:
                    for m in range(7):
                        t_mm2(m)
                    transp(0); transp(1)
                    t_mm2(7)
                    c_mm1(0)
                    transp(2); transp(3)
                    c_mm1(1)
                    transp(4); transp(5)
                    c_mm1(2)
                    transp(6); transp(7)
                    c_mm1(3)
                elif CG == 2:
                    for m in range(6):
                        t_mm2(m)
                    transp(0); transp(1)
                    t_mm2(6)
                    transp(2); transp(3)
                    t_mm2(7)
                    c_mm1(0)
                    transp(4); transp(5); transp(6); transp(7)
                    c_mm1(1)
                else:
                    for m in range(6):
                        t_mm2(m)
                    cw2t = cw2_prefetch()
                    transp(0); transp(1)
                    t_mm2(6)
                    transp(2); transp(3)
                    t_mm2(7)
                    for r in range(4, RT):
                        transp(r)
                    c_mm1(0)

                # hoist next batch's LN1 stats+apply into this channel phase
                if b + 1 < bpc:
                    y_next = stats_apply(x_next)
                    pending[0] = (x_next, y_next)

                # ---- channel MM2 + bias + residual -> out ----
                for mt in range(RT):
                    pns = psum.tile([128, H], F32, name="pns", tag="ps", bufs=2)
                    for ko in range(RT):
                        for n in range(2):
                            nc.tensor.matmul(
                                pns[:, 512 * n:512 * n + 512],
                                v1g[:, ko, 128 * mt:128 * mt + 128],
                                cw2_sb[:, ko, 512 * n:512 * n + 512],
                                start=(ko == 0), stop=(ko == RT - 1))
                    o_t = otp.tile([128, H], F32, name="o_t", tag="o")
                    nc.vector.scalar_tensor_tensor(
                        out=o_t[:], in0=pns[:], scalar=1.0,
                        in1=x_sb[:, mt, :], op0=ALU.mult, op1=ALU.add)
                    nc.gpsimd.tensor_add(o_t[:], o_t[:], cb2_sb[:])
                    nc.sync.dma_start(out_d[b][:, mt, :], o_t[:])

            def rep_body():
                x0 = load_x(0, eng=nc.scalar)
                y0 = stats_apply(x0)
                pending[0] = (x0, y0)
                for b in range(bpc):
                    batch_body(b)

            if time_reps > 1 and not unroll_reps:
                assert time_reps % body_reps == 0
                with tc.For_i(0, time_reps // body_reps, 1,
                              hint_engines=(mybir.EngineType.PE,
                                            mybir.EngineType.DVE,
                                            mybir.EngineType.Activation,
                                            mybir.EngineType.SP,
                                            mybir.EngineType.Pool)):
                    for _ in range(body_reps):
                        rep_body()
            else:
                for _ in range(time_reps):
                    rep_body()

    nc.compile()
    return nc


def prep_inputs(x, tw1, tb1, tw2, tb2, cw1, cb1, cw2, cb2,
                ln1_g, ln1_b, ln2_g, ln2_b):
    """Host-side layout + weight folding. Returns (in_maps, apply_g1, apply_b1)."""
    f = np.float32
    x = np.ascontiguousarray(np.asarray(x, f))
    mask = np.tril(np.ones((T, T), f))
    w1mT = (mask * np.asarray(tw1, f)).T          # [j, i]
    w2mT = (mask * np.asarray(tw2, f)).T
    cw1 = np.asarray(cw1, f)
    cw2 = np.asarray(cw2, f)
    ln2_g = np.asarray(ln2_g, f)
    ln2_b = np.asarray(ln2_b, f)
    # fold LN2 affine into channel MLP first layer
    cw1p = cw1 * ln2_g[None, :]                   # [o, h]
    cb1p = np.asarray(cb1, f) + cw1 @ ln2_b       # [o]
    cw1pT = cw1p.T                                # [h, o]
    cw2T = cw2.T                                  # [o, p]

    def tiles4(w):   # [1024,1024] -> [128, 8(k), 8(m), 128] (p=row%128)
        return w.reshape(RT, 128, RT, 128).transpose(1, 0, 2, 3)

    def tri_pack(w):  # -> [128, 36, 128] bf16, slot TRI[m]+k for k<=m
        t4 = tiles4(w)
        return np.ascontiguousarray(np.stack(
            [t4[:, k, m] for m in range(RT) for k in range(m + 1)],
            axis=1).astype(_bf16))

    def full_pack(w):  # -> [128, 64, 128] bf16, slot mo*8+kh
        t4 = tiles4(w)
        return np.ascontiguousarray(np.stack(
            [t4[:, kh, mo] for mo in range(RT) for kh in range(RT)],
            axis=1).astype(_bf16))

    def tiles3(w):   # [1024,1024] -> [128, 8, 1024]
        return np.ascontiguousarray(
            w.reshape(RT, 128, H).transpose(1, 0, 2).astype(_bf16))

    def bias_t(v):   # [1024] -> [128, 8]
        return np.ascontiguousarray(np.asarray(v, f).reshape(RT, 128).T)

    g1 = np.asarray(ln1_g, f)
    b1 = np.asarray(ln1_b, f)
    apply_g1 = not np.all(g1 == 1.0)
    apply_b1 = not np.all(b1 == 0.0)

    shared = {
        "w1": tri_pack(w1mT), "w2": tri_pack(w2mT),
        "cw1": full_pack(cw1pT), "cw2": tiles3(cw2T),
        "tb1": bias_t(tb1), "tb2": bias_t(tb2), "cb1": bias_t(cb1p),
        "cb2": np.ascontiguousarray(np.asarray(cb2, f)),
        "g1": np.ascontiguousarray(g1), "b1": np.ascontiguousarray(b1),
    }
    # x: [B,T,H] -> per-core [BPC, 128, RT, H]  (t = r*128 + p), bf16
    xs = x.reshape(NCORES, BPC, RT, 128, H).transpose(0, 1, 3, 2, 4).astype(_bf16)
    in_maps = [{"x": np.ascontiguousarray(xs[c]), **shared}
               for c in range(NCORES)]
    return in_maps, apply_g1, apply_b1


_cache = {}


def kernel(**inputs) -> np.ndarray:
    in_maps, apply_g1, apply_b1 = prep_inputs(**inputs)
    key = (apply_g1, apply_b1)
    if key not in _cache:
        _cache[key] = build(apply_g1=apply_g1, apply_b1=apply_b1, time_reps=1)
    nc = _cache[key]
    res = run_bass_kernel_spmd(nc, in_maps, list(range(NCORES)))
    # out per core: [BPC, 128, RT, H] -> [BPC, T, H]
    outs = [np.asarray(r["out"]).astype(np.float32)
            .transpose(0, 2, 1, 3).reshape(BPC, T, H) for r in res.results]
    return np.ascontiguousarray(np.concatenate(outs, axis=0), dtype=np.float32)
